# revision 7
# baseline (speedup 1.0000x reference)
"""Multi-head attention (B=2, S=2048, H=1024, 16 heads x 64d) on 8 trn2 cores.

Sharding: tensor-parallel over heads (2 heads/core). Each core computes the
qkv projection for its 384 output features, attention for its 2 heads, and a
partial o_proj ([4096,1024] over its 128-feature slice). Host sums the 8
partials and adds b_o.

Device layout (per core, feature-major):
  QT/KT [128, 4096]  rows = head_local*64 + d, cols = b*2048 + s  (fp32r)
  S^T orientation for scores ([k, q]) so softmax-sum over k falls out of the
  PV matmul via a ones-column appended to V; exp runs on ScalarE from PSUM;
  normalization = reciprocal of the sums row + ones-broadcast matmul + mul.
All matmuls run as float32r (1 cyc/row vs 4 for fp32, ~1e-4 rel err).
"""
import sys

sys.path.insert(0, "/opt/trn_rl_repo")
import numpy as np

NHEADS = 16
HEAD_DIM = 64
HIDDEN = 1024
QKV = NHEADS * HEAD_DIM  # 1024
SCALING = HEAD_DIM ** -0.5
B = 2
S = 2048
T = B * S  # 4096
NCORES = 8
HPC = NHEADS // NCORES  # 2 heads per core
FEAT = HPC * HEAD_DIM  # 128
CHUNK = 512
NCHUNK = S // CHUNK  # 4 per batch
KSLABS = HIDDEN // 128  # 8
SSLABS = S // 128  # 16
D1 = HEAD_DIM + 1  # 65

_CACHE = {}
LAST_RESULT = None  # BassKernelResults of the most recent kernel() call


def _split_waits(nc, keep=1):
    """Hoist excess per-instruction sem waits into standalone EventSemaphore
    instructions (walrus codegen has small per-opcode wait budgets)."""
    import bass_rust
    import concourse.mybir as mybir

    n_new = 0
    for f in nc.m.functions:
        for blk in f.blocks:
            out = []
            changed = False
            for inst in blk.instructions:
                si = inst.sync_info
                waits = list(si.on_wait) if si is not None else []
                if len(waits) > keep:
                    excess = waits[: len(waits) - keep]
                    kept = waits[len(waits) - keep:]
                    for w in excess:
                        out.append(mybir.InstEventSemaphore(
                            name=f"{inst.name}-esw{n_new}",
                            engine=inst.engine,
                            sync_info=bass_rust.SyncInfo(on_wait=[w], on_update=[]),
                        ))
                        n_new += 1
                    inst.sync_info = bass_rust.SyncInfo(
                        on_wait=kept, on_update=list(si.on_update))
                    changed = True
                out.append(inst)
            if changed:
                blk.instructions = out
    return n_new


def _build(reps=1):
    import concourse.bass as bass
    import concourse.mybir as mybir
    import concourse.tile as tile
    from concourse.masks import make_identity

    f32 = mybir.dt.float32
    f32r = mybir.dt.float32r
    Exp = mybir.ActivationFunctionType.Exp

    nc = bass.Bass()
    xT = nc.dram_tensor("xT", [HIDDEN, T], f32r, kind="ExternalInput")
    wqkvT = nc.dram_tensor("wqkvT", [HIDDEN, 3 * FEAT], f32r, kind="ExternalInput")
    bqkv = nc.dram_tensor("bqkv", [FEAT, 3], f32, kind="ExternalInput")
    woT = nc.dram_tensor("woT", [FEAT, HIDDEN], f32r, kind="ExternalInput")
    out_d = nc.dram_tensor("out", [T, HIDDEN], f32, kind="ExternalOutput")

    with tile.TileContext(nc) as tc, nc.allow_low_precision(reason="fp32r matmuls"):
        with (
            tc.tile_pool(name="sing", bufs=1) as sing,
            tc.tile_pool(name="xp", bufs=2) as xp,
            tc.tile_pool(name="pp", bufs=2) as pp,
            tc.tile_pool(name="stg", bufs=4) as stg,
            tc.tile_pool(name="sm", bufs=2) as sm,
            tc.tile_pool(name="ps_mm", bufs=2, space="PSUM") as ps_mm,
            tc.tile_pool(name="ps_s", bufs=2, space="PSUM") as ps_s,
            tc.tile_pool(name="ps_o", bufs=2, space="PSUM") as ps_o,
        ):
            wq_sb = sing.tile([128, KSLABS, 3 * FEAT], f32r, tag="wq")
            wo_sb = sing.tile([FEAT, HIDDEN], f32r, tag="wo")
            bq_sb = sing.tile([FEAT, 3], f32, tag="bq")
            ident = sing.tile([128, 128], f32, tag="id")
            ones1 = sing.tile([1, HEAD_DIM], f32r, tag="on")
            QT = sing.tile([128, T], f32r, tag="qt")
            KT = sing.tile([128, T], f32r, tag="kt")
            VT = sing.tile([128, T], f32, tag="vt")
            OT = sing.tile([128, T], f32r, tag="ot")
            Vaug = sing.tile([128, B, HPC, SSLABS, D1], f32r, tag="va")

            nc.sync.dma_start(
                out=wq_sb, in_=wqkvT[:].rearrange("(s p) f -> p s f", p=128))
            nc.sync.dma_start(out=wo_sb, in_=woT[:])
            nc.sync.dma_start(out=bq_sb, in_=bqkv[:])
            make_identity(nc, ident)
            ones_f = stg.tile([1, HEAD_DIM], f32, tag="onf")
            nc.vector.memset(ones_f, 1.0)
            nc.vector.tensor_copy(ones1, ones_f)
            vst = stg.tile([128, B * HPC * SSLABS], f32, tag="vst")
            nc.vector.memset(vst, 1.0)
            nc.vector.tensor_copy(Vaug[:, :, :, :, HEAD_DIM:D1], vst)

            xT_r = xT[:].rearrange("(s p) t -> s p t", p=128)

            def qkv_chunk(b, n):
                g = b * NCHUNK + n
                lo, hi = g * CHUNK, (g + 1) * CHUNK
                xs = []
                for s in range(KSLABS):
                    xt_ = xp.tile([128, CHUNK], f32r, tag=f"x{s}")
                    nc.sync.dma_start(out=xt_, in_=xT_r[s, :, lo:hi])
                    xs.append(xt_)
                for m, dest in enumerate((QT, KT, VT)):
                    acc = ps_mm.tile([128, CHUNK], f32, tag="mm")
                    for s in range(KSLABS):
                        nc.tensor.matmul(
                            acc, wq_sb[:, s, m * FEAT:(m + 1) * FEAT], xs[s],
                            start=(s == 0), stop=(s == KSLABS - 1))
                    nc.vector.tensor_scalar_add(
                        dest[:, lo:hi], acc, bq_sb[:, m:m + 1])

            def vtrans(b, n):
                # transpose V token-slabs 4n..4n+3 (one qkv chunk's worth)
                for h in range(HPC):
                    for k in range(4 * n, 4 * n + 4):
                        tp = ps_mm.tile([128, HEAD_DIM], f32, tag="mm")
                        nc.tensor.transpose(
                            tp,
                            VT[64 * h:64 * h + 64,
                               b * S + 128 * k: b * S + 128 * (k + 1)],
                            ident[64 * h:64 * h + 64, 64 * h:64 * h + 64])
                        nc.vector.tensor_copy(Vaug[:, b, h, k, 0:HEAD_DIM], tp)

            GRP = 2  # S-slabs per exp group (ps_s holds GRP banks x 2 bufs)

            def attn_unit(b, h, qc):
                qlo = b * S + qc * CHUNK
                qsl = slice(qlo, qlo + CHUNK)
                o_ps = ps_o.tile([D1, CHUNK], f32, tag="o")
                for grp in range(SSLABS // GRP):
                    s_ps = ps_s.tile([128, GRP, CHUNK], f32, tag="s")
                    for kk in range(GRP):
                        k = grp * GRP + kk
                        nc.tensor.matmul(
                            s_ps[:, kk, :],
                            KT[64 * h:64 * h + 64,
                               b * S + 128 * k: b * S + 128 * (k + 1)],
                            QT[64 * h:64 * h + 64, qsl],
                            start=True, stop=True)
                    pt = pp.tile([128, GRP, CHUNK], f32r, tag="pt")
                    nc.scalar.activation(out=pt, in_=s_ps, func=Exp)
                    for kk in range(GRP):
                        k = grp * GRP + kk
                        nc.tensor.matmul(
                            o_ps, Vaug[:, b, h, k, :], pt[:, kk, :],
                            start=(k == 0), stop=(k == SSLABS - 1))
                rec = sm.tile([1, CHUNK], f32r, tag="rec")
                nc.vector.reciprocal(rec, o_ps[HEAD_DIM:D1, :])
                b_ps = ps_mm.tile([HEAD_DIM, CHUNK], f32, tag="mm")
                nc.tensor.matmul(b_ps, ones1, rec, start=True, stop=True)
                rb = sm.tile([HEAD_DIM, CHUNK], f32, tag="rb")
                nc.vector.tensor_copy(rb, b_ps)
                nc.vector.tensor_mul(
                    OT[64 * h:64 * h + 64, qsl], o_ps[0:HEAD_DIM, :], rb)

            def oproj_tile(t):
                for nh in range(HIDDEN // CHUNK):
                    acc = ps_mm.tile([128, CHUNK], f32, tag="mm")
                    nc.tensor.matmul(
                        acc, OT[:, 128 * t:128 * (t + 1)],
                        wo_sb[:, nh * CHUNK:(nh + 1) * CHUNK],
                        start=True, stop=True)
                    ost = stg.tile([128, CHUNK], f32, tag="ost")
                    nc.vector.tensor_copy(ost, acc)
                    nc.sync.dma_start(
                        out=out_d[128 * t:128 * (t + 1),
                                  nh * CHUNK:(nh + 1) * CHUNK],
                        in_=ost)

            # ---- emission order: overlap qkv(b1) / oproj(b0) with attention ----
            for _rep in range(reps):
                for n in range(NCHUNK):
                    qkv_chunk(0, n)
                    vtrans(0, n)
                units_b0 = [(0, h, qc) for h in range(HPC) for qc in range(NCHUNK)]
                for i, (b, h, qc) in enumerate(units_b0):
                    attn_unit(b, h, qc)
                    if i < NCHUNK:
                        qkv_chunk(1, i)
                        vtrans(1, i)
                units_b1 = [(1, h, qc) for h in range(HPC) for qc in range(NCHUNK)]
                for i, (b, h, qc) in enumerate(units_b1):
                    attn_unit(b, h, qc)
                    oproj_tile(2 * i)      # batch-0 tiles 0..15
                    oproj_tile(2 * i + 1)
                    if h == 1:             # batch-1 tiles as soon as both heads done
                        for t in range(16 + 4 * qc, 20 + 4 * qc):
                            oproj_tile(t)

    _split_waits(nc)
    return nc


def kernel(hidden_states, w_qkv, b_qkv, w_o, b_o):
    global LAST_RESULT
    from concourse.bass_utils import run_bass_kernel_spmd
    import os

    if "nc" not in _CACHE:
        _CACHE["nc"] = _build()
    nc = _CACHE["nc"]

    x = np.ascontiguousarray(
        np.asarray(hidden_states, dtype=np.float32).reshape(T, HIDDEN).T)
    w_qkv = np.asarray(w_qkv, dtype=np.float32)
    b_qkv = np.asarray(b_qkv, dtype=np.float32)
    w_o = np.asarray(w_o, dtype=np.float32)
    b_o = np.asarray(b_o, dtype=np.float32)

    in_maps = []
    for c in range(NCORES):
        rq = slice(c * FEAT, (c + 1) * FEAT)
        wq = w_qkv[0:QKV][rq] * SCALING
        wk = w_qkv[QKV:2 * QKV][rq]
        wv = w_qkv[2 * QKV:3 * QKV][rq]
        bq = b_qkv[0:QKV][rq] * SCALING
        bk = b_qkv[QKV:2 * QKV][rq]
        bv = b_qkv[2 * QKV:3 * QKV][rq]
        in_maps.append({
            "xT": x,
            "wqkvT": np.ascontiguousarray(
                np.concatenate([wq, wk, wv], axis=0).T),
            "bqkv": np.ascontiguousarray(np.stack([bq, bk, bv], axis=1)),
            "woT": np.ascontiguousarray(w_o[:, rq].T),
        })

    trace = bool(os.environ.get("KERNEL_TRACE"))
    res = run_bass_kernel_spmd(nc, in_maps, list(range(NCORES)), trace=trace)
    LAST_RESULT = res

    acc = np.zeros((T, HIDDEN), dtype=np.float64)
    for c in range(NCORES):
        acc += res.results[c]["out"]
    out = (acc + b_o).astype(np.float32).reshape(B, S, HIDDEN)
    return out


# revision 11
# speedup vs baseline: 1.0290x; 1.0290x over previous
"""Multi-head attention (B=2, S=2048, H=1024, 16 heads x 64d) on 8 trn2 cores.

Sharding: tensor-parallel over heads (2 heads/core). Each core computes the
qkv projection for its 384 output features, attention for its 2 heads, and a
partial o_proj ([4096,1024] over its 128-feature slice). Host sums the 8
partials and adds b_o.

Device layout (per core, feature-major):
  QT/KT [128, 4096]  rows = head_local*64 + d, cols = b*2048 + s  (fp32r)
  S^T orientation for scores ([k, q]) so softmax-sum over k falls out of the
  PV matmul via a ones-column appended to V; exp runs on ScalarE from PSUM;
  normalization = reciprocal of the sums row + ones-broadcast matmul + mul.
All matmuls run as float32r (1 cyc/row vs 4 for fp32, ~1e-4 rel err).
"""
import sys

sys.path.insert(0, "/opt/trn_rl_repo")
import numpy as np

NHEADS = 16
HEAD_DIM = 64
HIDDEN = 1024
QKV = NHEADS * HEAD_DIM  # 1024
SCALING = HEAD_DIM ** -0.5
B = 2
S = 2048
T = B * S  # 4096
NCORES = 8
HPC = NHEADS // NCORES  # 2 heads per core
FEAT = HPC * HEAD_DIM  # 128
CHUNK = 512
NCHUNK = S // CHUNK  # 4 per batch
KSLABS = HIDDEN // 128  # 8
SSLABS = S // 128  # 16
D1 = HEAD_DIM + 1  # 65

_CACHE = {}
LAST_RESULT = None  # BassKernelResults of the most recent kernel() call


def _split_waits(nc, keep=1):
    """Hoist excess per-instruction sem waits into standalone EventSemaphore
    instructions (walrus codegen has small per-opcode wait budgets)."""
    import bass_rust
    import concourse.mybir as mybir

    n_new = 0
    for f in nc.m.functions:
        for blk in f.blocks:
            out = []
            changed = False
            for inst in blk.instructions:
                si = inst.sync_info
                waits = list(si.on_wait) if si is not None else []
                if len(waits) > keep:
                    excess = waits[: len(waits) - keep]
                    kept = waits[len(waits) - keep:]
                    for w in excess:
                        out.append(mybir.InstEventSemaphore(
                            name=f"{inst.name}-esw{n_new}",
                            engine=inst.engine,
                            sync_info=bass_rust.SyncInfo(on_wait=[w], on_update=[]),
                        ))
                        n_new += 1
                    inst.sync_info = bass_rust.SyncInfo(
                        on_wait=kept, on_update=list(si.on_update))
                    changed = True
                out.append(inst)
            if changed:
                blk.instructions = out
    return n_new


def _build(reps=1):
    import concourse.bass as bass
    import concourse.mybir as mybir
    import concourse.tile as tile
    from concourse.masks import make_identity

    f32 = mybir.dt.float32
    f32r = mybir.dt.float32r
    Exp = mybir.ActivationFunctionType.Exp

    nc = bass.Bass()
    xT = nc.dram_tensor("xT", [HIDDEN, T], f32r, kind="ExternalInput")
    wqkvT = nc.dram_tensor("wqkvT", [HIDDEN, 3 * FEAT], f32r, kind="ExternalInput")
    bqkv = nc.dram_tensor("bqkv", [FEAT, 3], f32, kind="ExternalInput")
    woT = nc.dram_tensor("woT", [FEAT, HIDDEN], f32r, kind="ExternalInput")
    out_d = nc.dram_tensor("out", [T, HIDDEN], f32, kind="ExternalOutput")

    with tile.TileContext(nc) as tc, nc.allow_low_precision(reason="fp32r matmuls"):
        with (
            tc.tile_pool(name="sing", bufs=1) as sing,
            tc.tile_pool(name="xp", bufs=2) as xp,
            tc.tile_pool(name="pp", bufs=2) as pp,
            tc.tile_pool(name="stg", bufs=4) as stg,
            tc.tile_pool(name="sm", bufs=2) as sm,
            tc.tile_pool(name="op", bufs=2) as op,
            tc.tile_pool(name="ps_mm", bufs=2, space="PSUM") as ps_mm,
            tc.tile_pool(name="ps_s", bufs=2, space="PSUM") as ps_s,
            tc.tile_pool(name="ps_o", bufs=2, space="PSUM") as ps_o,
        ):
            wq_sb = sing.tile([128, KSLABS, 3 * FEAT], f32r, tag="wq")
            wo_sb = sing.tile([FEAT, HIDDEN], f32r, tag="wo")
            bq_sb = sing.tile([FEAT, 3], f32, tag="bq")
            ident = sing.tile([128, 128], f32, tag="id")
            ones1 = sing.tile([1, HEAD_DIM], f32r, tag="on")
            QT = sing.tile([128, T], f32r, tag="qt")
            KT = sing.tile([128, T], f32r, tag="kt")
            VT = sing.tile([128, T], f32, tag="vt")
            OT = sing.tile([128, T], f32r, tag="ot")
            Vaug = sing.tile([128, B, HPC, SSLABS, D1], f32r, tag="va")

            nc.sync.dma_start(
                out=wq_sb, in_=wqkvT[:].rearrange("(s p) f -> p s f", p=128))
            nc.sync.dma_start(out=wo_sb, in_=woT[:])
            nc.sync.dma_start(out=bq_sb, in_=bqkv[:])
            make_identity(nc, ident)
            ones_f = stg.tile([1, HEAD_DIM], f32, tag="onf")
            nc.vector.memset(ones_f, 1.0)
            nc.vector.tensor_copy(ones1, ones_f)
            vst = stg.tile([128, B * HPC * SSLABS], f32, tag="vst")
            nc.vector.memset(vst, 1.0)
            nc.vector.tensor_copy(Vaug[:, :, :, :, HEAD_DIM:D1], vst)

            xT_r = xT[:].rearrange("(s p) t -> s p t", p=128)

            xT_c = xT[:].rearrange("(s p) t -> p s t", p=128)

            def qkv_chunk(b, n):
                g = b * NCHUNK + n
                lo, hi = g * CHUNK, (g + 1) * CHUNK
                xc = xp.tile([128, KSLABS, CHUNK], f32r, tag="xc")
                nc.sync.dma_start(out=xc, in_=xT_c[:, :, lo:hi])
                for m, dest in enumerate((QT, KT, VT)):
                    acc = ps_mm.tile([128, CHUNK], f32, tag="mm")
                    for s in range(KSLABS):
                        nc.tensor.matmul(
                            acc, wq_sb[:, s, m * FEAT:(m + 1) * FEAT], xc[:, s, :],
                            start=(s == 0), stop=(s == KSLABS - 1))
                    nc.vector.tensor_scalar_add(
                        dest[:, lo:hi], acc, bq_sb[:, m:m + 1])

            def vtrans(b, n):
                # transpose V token-slabs 4n..4n+3 (one qkv chunk's worth)
                for h in range(HPC):
                    for k in range(4 * n, 4 * n + 4):
                        tp = ps_mm.tile([128, HEAD_DIM], f32, tag="mm")
                        nc.tensor.transpose(
                            tp,
                            VT[64 * h:64 * h + 64,
                               b * S + 128 * k: b * S + 128 * (k + 1)],
                            ident[64 * h:64 * h + 64, 64 * h:64 * h + 64])
                        nc.vector.tensor_copy(Vaug[:, b, h, k, 0:HEAD_DIM], tp)

            GRP = 2  # S-slabs per exp group (ps_s holds GRP banks x 2 bufs)

            def attn_unit(b, h, qc):
                qlo = b * S + qc * CHUNK
                qsl = slice(qlo, qlo + CHUNK)
                o_ps = ps_o.tile([D1, CHUNK], f32, tag="o")
                for grp in range(SSLABS // GRP):
                    s_ps = ps_s.tile([128, GRP, CHUNK], f32, tag="s")
                    for kk in range(GRP):
                        k = grp * GRP + kk
                        nc.tensor.matmul(
                            s_ps[:, kk, :],
                            KT[64 * h:64 * h + 64,
                               b * S + 128 * k: b * S + 128 * (k + 1)],
                            QT[64 * h:64 * h + 64, qsl],
                            start=True, stop=True)
                    pt = pp.tile([128, GRP, CHUNK], f32r, tag="pt")
                    nc.scalar.activation(out=pt, in_=s_ps, func=Exp)
                    for kk in range(GRP):
                        k = grp * GRP + kk
                        nc.tensor.matmul(
                            o_ps, Vaug[:, b, h, k, :], pt[:, kk, :],
                            start=(k == 0), stop=(k == SSLABS - 1))
                rec = sm.tile([1, CHUNK], f32r, tag="rec")
                nc.vector.reciprocal(rec, o_ps[HEAD_DIM:D1, :])
                b_ps = ps_mm.tile([HEAD_DIM, CHUNK], f32, tag="mm")
                nc.tensor.matmul(b_ps, ones1, rec, start=True, stop=True)
                rb = sm.tile([HEAD_DIM, CHUNK], f32, tag="rb")
                nc.vector.tensor_copy(rb, b_ps)
                nc.vector.tensor_mul(
                    OT[64 * h:64 * h + 64, qsl], o_ps[0:HEAD_DIM, :], rb)

            def oproj_group(j):
                # token tiles 4j..4j+3 (tokens 512j..512j+512), one out-DMA
                ost = op.tile([128, 4, HIDDEN], f32, tag="ost")
                for jj in range(4):
                    t = 4 * j + jj
                    for nh in range(HIDDEN // CHUNK):
                        acc = ps_mm.tile([128, CHUNK], f32, tag="mm")
                        nc.tensor.matmul(
                            acc, OT[:, 128 * t:128 * (t + 1)],
                            wo_sb[:, nh * CHUNK:(nh + 1) * CHUNK],
                            start=True, stop=True)
                        nc.vector.tensor_copy(
                            ost[:, jj, nh * CHUNK:(nh + 1) * CHUNK], acc)
                nc.sync.dma_start(
                    out=out_d[512 * j:512 * (j + 1), :].rearrange(
                        "(jj p) h -> p jj h", p=128),
                    in_=ost)

            # ---- emission order: overlap qkv(b1) / oproj(b0) with attention ----
            for _rep in range(reps):
                for n in range(NCHUNK):
                    qkv_chunk(0, n)
                    vtrans(0, n)
                units_b0 = [(0, h, qc) for h in range(HPC) for qc in range(NCHUNK)]
                for i, (b, h, qc) in enumerate(units_b0):
                    attn_unit(b, h, qc)
                    if i < NCHUNK:
                        qkv_chunk(1, i)
                        vtrans(1, i)
                units_b1 = [(1, h, qc) for h in range(HPC) for qc in range(NCHUNK)]
                for i, (b, h, qc) in enumerate(units_b1):
                    attn_unit(b, h, qc)
                    if h == 0:             # batch-0 groups 0..3
                        oproj_group(qc)
                    else:                  # batch-1 group qc ready after both heads
                        oproj_group(4 + qc)

    _split_waits(nc)
    return nc


def kernel(hidden_states, w_qkv, b_qkv, w_o, b_o):
    global LAST_RESULT
    from concourse.bass_utils import run_bass_kernel_spmd
    import os

    if "nc" not in _CACHE:
        _CACHE["nc"] = _build()
    nc = _CACHE["nc"]

    x = np.ascontiguousarray(
        np.asarray(hidden_states, dtype=np.float32).reshape(T, HIDDEN).T)
    w_qkv = np.asarray(w_qkv, dtype=np.float32)
    b_qkv = np.asarray(b_qkv, dtype=np.float32)
    w_o = np.asarray(w_o, dtype=np.float32)
    b_o = np.asarray(b_o, dtype=np.float32)

    in_maps = []
    for c in range(NCORES):
        rq = slice(c * FEAT, (c + 1) * FEAT)
        wq = w_qkv[0:QKV][rq] * SCALING
        wk = w_qkv[QKV:2 * QKV][rq]
        wv = w_qkv[2 * QKV:3 * QKV][rq]
        bq = b_qkv[0:QKV][rq] * SCALING
        bk = b_qkv[QKV:2 * QKV][rq]
        bv = b_qkv[2 * QKV:3 * QKV][rq]
        in_maps.append({
            "xT": x,
            "wqkvT": np.ascontiguousarray(
                np.concatenate([wq, wk, wv], axis=0).T),
            "bqkv": np.ascontiguousarray(np.stack([bq, bk, bv], axis=1)),
            "woT": np.ascontiguousarray(w_o[:, rq].T),
        })

    trace = bool(os.environ.get("KERNEL_TRACE"))
    res = run_bass_kernel_spmd(nc, in_maps, list(range(NCORES)), trace=trace)
    LAST_RESULT = res

    acc = np.zeros((T, HIDDEN), dtype=np.float64)
    for c in range(NCORES):
        acc += res.results[c]["out"]
    out = (acc + b_o).astype(np.float32).reshape(B, S, HIDDEN)
    return out


# revision 13
# speedup vs baseline: 1.2353x; 1.2004x over previous
"""Multi-head attention (B=2, S=2048, H=1024, 16 heads x 64d) on 8 trn2 cores.

Sharding: tensor-parallel over heads (2 heads/core). Each core computes the
qkv projection for its 384 output features, attention for its 2 heads, and a
partial o_proj ([4096,1024] over its 128-feature slice). Host sums the 8
partials and adds b_o.

Device layout (per core, feature-major):
  QT/KT [128, 4096]  rows = head_local*64 + d, cols = b*2048 + s  (fp32r)
  S^T orientation for scores ([k, q]) so softmax-sum over k falls out of the
  PV matmul via a ones-column appended to V; exp runs on ScalarE from PSUM;
  normalization = reciprocal of the sums row + ones-broadcast matmul + mul.
All matmuls run as float32r (1 cyc/row vs 4 for fp32, ~1e-4 rel err).
"""
import sys

sys.path.insert(0, "/opt/trn_rl_repo")
import numpy as np

NHEADS = 16
HEAD_DIM = 64
HIDDEN = 1024
QKV = NHEADS * HEAD_DIM  # 1024
SCALING = HEAD_DIM ** -0.5
B = 2
S = 2048
T = B * S  # 4096
NCORES = 8
HPC = NHEADS // NCORES  # 2 heads per core
FEAT = HPC * HEAD_DIM  # 128
CHUNK = 512
NCHUNK = S // CHUNK  # 4 per batch
KSLABS = HIDDEN // 128  # 8
SSLABS = S // 128  # 16
D1 = HEAD_DIM + 1  # 65

_CACHE = {}
LAST_RESULT = None  # BassKernelResults of the most recent kernel() call


def _split_waits(nc, keep=1):
    """Hoist excess per-instruction sem waits into standalone EventSemaphore
    instructions (walrus codegen has small per-opcode wait budgets)."""
    import bass_rust
    import concourse.mybir as mybir

    n_new = 0
    for f in nc.m.functions:
        for blk in f.blocks:
            out = []
            changed = False
            for inst in blk.instructions:
                si = inst.sync_info
                waits = list(si.on_wait) if si is not None else []
                if len(waits) > keep:
                    excess = waits[: len(waits) - keep]
                    kept = waits[len(waits) - keep:]
                    for w in excess:
                        out.append(mybir.InstEventSemaphore(
                            name=f"{inst.name}-esw{n_new}",
                            engine=inst.engine,
                            sync_info=bass_rust.SyncInfo(on_wait=[w], on_update=[]),
                        ))
                        n_new += 1
                    inst.sync_info = bass_rust.SyncInfo(
                        on_wait=kept, on_update=list(si.on_update))
                    changed = True
                out.append(inst)
            if changed:
                blk.instructions = out
    return n_new


def _build(reps=1):
    import concourse.bass as bass
    import concourse.mybir as mybir
    import concourse.tile as tile
    from concourse.masks import make_identity

    f32 = mybir.dt.float32
    f32r = mybir.dt.float32r
    f16 = mybir.dt.float16
    Exp = mybir.ActivationFunctionType.Exp

    nc = bass.Bass()
    xT = nc.dram_tensor("xT", [HIDDEN, T], f32r, kind="ExternalInput")
    wqkvT = nc.dram_tensor("wqkvT", [HIDDEN, 3 * FEAT], f32r, kind="ExternalInput")
    bqkv = nc.dram_tensor("bqkv", [FEAT, 3], f32, kind="ExternalInput")
    woT = nc.dram_tensor("woT", [FEAT, HIDDEN], f32r, kind="ExternalInput")
    out_d = nc.dram_tensor("out", [T, HIDDEN], f32, kind="ExternalOutput")

    with tile.TileContext(nc) as tc, nc.allow_low_precision(reason="fp32r matmuls"):
        with (
            tc.tile_pool(name="sing", bufs=1) as sing,
            tc.tile_pool(name="xp", bufs=2) as xp,
            tc.tile_pool(name="pp", bufs=2) as pp,
            tc.tile_pool(name="stg", bufs=4) as stg,
            tc.tile_pool(name="sm", bufs=2) as sm,
            tc.tile_pool(name="op", bufs=2) as op,
            tc.tile_pool(name="ps_mm", bufs=2, space="PSUM") as ps_mm,
            tc.tile_pool(name="ps_s", bufs=2, space="PSUM") as ps_s,
            tc.tile_pool(name="ps_o", bufs=2, space="PSUM") as ps_o,
        ):
            wq_sb = sing.tile([128, KSLABS, 3 * FEAT], f32r, tag="wq")
            wo_sb = sing.tile([FEAT, HIDDEN], f32r, tag="wo")
            bq_sb = sing.tile([FEAT, 3], f32, tag="bq")
            ident = sing.tile([128, 128], f32, tag="id")
            ones1 = sing.tile([1, HEAD_DIM], f32r, tag="on")
            QT = sing.tile([128, T], f16, tag="qt")
            KT = sing.tile([128, T], f16, tag="kt")
            VT = sing.tile([128, T], f32, tag="vt")
            OT = sing.tile([128, T], f32r, tag="ot")
            Vaug = sing.tile([128, B, HPC, SSLABS, D1], f16, tag="va")

            nc.sync.dma_start(
                out=wq_sb, in_=wqkvT[:].rearrange("(s p) f -> p s f", p=128))
            nc.sync.dma_start(out=wo_sb, in_=woT[:])
            nc.sync.dma_start(out=bq_sb, in_=bqkv[:])
            make_identity(nc, ident)
            ones_f = stg.tile([1, HEAD_DIM], f32, tag="onf")
            nc.vector.memset(ones_f, 1.0)
            nc.vector.tensor_copy(ones1, ones_f)
            vst = stg.tile([128, B * HPC * SSLABS], f32, tag="vst")
            nc.vector.memset(vst, 1.0)
            nc.vector.tensor_copy(Vaug[:, :, :, :, HEAD_DIM:D1], vst)

            xT_r = xT[:].rearrange("(s p) t -> s p t", p=128)

            xT_c = xT[:].rearrange("(s p) t -> p s t", p=128)

            def qkv_chunk(b, n):
                g = b * NCHUNK + n
                lo, hi = g * CHUNK, (g + 1) * CHUNK
                xc = xp.tile([128, KSLABS, CHUNK], f32r, tag="xc")
                nc.sync.dma_start(out=xc, in_=xT_c[:, :, lo:hi])
                for m, dest in enumerate((QT, KT, VT)):
                    acc = ps_mm.tile([128, CHUNK], f32, tag="mm")
                    for s in range(KSLABS):
                        nc.tensor.matmul(
                            acc, wq_sb[:, s, m * FEAT:(m + 1) * FEAT], xc[:, s, :],
                            start=(s == 0), stop=(s == KSLABS - 1))
                    nc.vector.tensor_scalar_add(
                        dest[:, lo:hi], acc, bq_sb[:, m:m + 1])

            def vtrans(b, n):
                # transpose V token-slabs 4n..4n+3 (one qkv chunk's worth)
                for h in range(HPC):
                    for k in range(4 * n, 4 * n + 4):
                        tp = ps_mm.tile([128, HEAD_DIM], f32, tag="mm")
                        nc.tensor.transpose(
                            tp,
                            VT[64 * h:64 * h + 64,
                               b * S + 128 * k: b * S + 128 * (k + 1)],
                            ident[64 * h:64 * h + 64, 64 * h:64 * h + 64])
                        nc.vector.tensor_copy(Vaug[:, b, h, k, 0:HEAD_DIM], tp)

            GRP = 2  # S-slabs per exp group (ps_s holds GRP banks x 2 bufs)

            def attn_unit(b, h, qc):
                qlo = b * S + qc * CHUNK
                qsl = slice(qlo, qlo + CHUNK)
                o_ps = ps_o.tile([D1, CHUNK], f32, tag="o")
                for grp in range(SSLABS // GRP):
                    s_ps = ps_s.tile([128, GRP, CHUNK], f32, tag="s")
                    for kk in range(GRP):
                        k = grp * GRP + kk
                        nc.tensor.matmul(
                            s_ps[:, kk, :],
                            KT[64 * h:64 * h + 64,
                               b * S + 128 * k: b * S + 128 * (k + 1)],
                            QT[64 * h:64 * h + 64, qsl],
                            start=True, stop=True)
                    pt = pp.tile([128, GRP, CHUNK], f16, tag="pt")
                    nc.scalar.activation(out=pt, in_=s_ps, func=Exp)
                    for kk in range(GRP):
                        k = grp * GRP + kk
                        nc.tensor.matmul(
                            o_ps, Vaug[:, b, h, k, :], pt[:, kk, :],
                            start=(k == 0), stop=(k == SSLABS - 1))
                rec = sm.tile([1, CHUNK], f32r, tag="rec")
                nc.vector.reciprocal(rec, o_ps[HEAD_DIM:D1, :])
                b_ps = ps_mm.tile([HEAD_DIM, CHUNK], f32, tag="mm")
                nc.tensor.matmul(b_ps, ones1, rec, start=True, stop=True)
                rb = sm.tile([HEAD_DIM, CHUNK], f32, tag="rb")
                nc.vector.tensor_copy(rb, b_ps)
                nc.vector.tensor_mul(
                    OT[64 * h:64 * h + 64, qsl], o_ps[0:HEAD_DIM, :], rb)

            def oproj_group(j):
                # token tiles 4j..4j+3 (tokens 512j..512j+512), one out-DMA
                ost = op.tile([128, 4, HIDDEN], f32, tag="ost")
                for jj in range(4):
                    t = 4 * j + jj
                    for nh in range(HIDDEN // CHUNK):
                        acc = ps_mm.tile([128, CHUNK], f32, tag="mm")
                        nc.tensor.matmul(
                            acc, OT[:, 128 * t:128 * (t + 1)],
                            wo_sb[:, nh * CHUNK:(nh + 1) * CHUNK],
                            start=True, stop=True)
                        nc.vector.tensor_copy(
                            ost[:, jj, nh * CHUNK:(nh + 1) * CHUNK], acc)
                nc.sync.dma_start(
                    out=out_d[512 * j:512 * (j + 1), :].rearrange(
                        "(jj p) h -> p jj h", p=128),
                    in_=ost)

            # ---- emission order: overlap qkv(b1) / oproj(b0) with attention ----
            for _rep in range(reps):
                for n in range(NCHUNK):
                    qkv_chunk(0, n)
                    vtrans(0, n)
                units_b0 = [(0, h, qc) for h in range(HPC) for qc in range(NCHUNK)]
                for i, (b, h, qc) in enumerate(units_b0):
                    attn_unit(b, h, qc)
                    if i < NCHUNK:
                        qkv_chunk(1, i)
                        vtrans(1, i)
                units_b1 = [(1, h, qc) for h in range(HPC) for qc in range(NCHUNK)]
                for i, (b, h, qc) in enumerate(units_b1):
                    attn_unit(b, h, qc)
                    if h == 0:             # batch-0 groups 0..3
                        oproj_group(qc)
                    else:                  # batch-1 group qc ready after both heads
                        oproj_group(4 + qc)

    _split_waits(nc)
    return nc


def kernel(hidden_states, w_qkv, b_qkv, w_o, b_o):
    global LAST_RESULT
    from concourse.bass_utils import run_bass_kernel_spmd
    import os

    if "nc" not in _CACHE:
        _CACHE["nc"] = _build()
    nc = _CACHE["nc"]

    x = np.ascontiguousarray(
        np.asarray(hidden_states, dtype=np.float32).reshape(T, HIDDEN).T)
    w_qkv = np.asarray(w_qkv, dtype=np.float32)
    b_qkv = np.asarray(b_qkv, dtype=np.float32)
    w_o = np.asarray(w_o, dtype=np.float32)
    b_o = np.asarray(b_o, dtype=np.float32)

    in_maps = []
    for c in range(NCORES):
        rq = slice(c * FEAT, (c + 1) * FEAT)
        wq = w_qkv[0:QKV][rq] * SCALING
        wk = w_qkv[QKV:2 * QKV][rq]
        wv = w_qkv[2 * QKV:3 * QKV][rq]
        bq = b_qkv[0:QKV][rq] * SCALING
        bk = b_qkv[QKV:2 * QKV][rq]
        bv = b_qkv[2 * QKV:3 * QKV][rq]
        in_maps.append({
            "xT": x,
            "wqkvT": np.ascontiguousarray(
                np.concatenate([wq, wk, wv], axis=0).T),
            "bqkv": np.ascontiguousarray(np.stack([bq, bk, bv], axis=1)),
            "woT": np.ascontiguousarray(w_o[:, rq].T),
        })

    trace = bool(os.environ.get("KERNEL_TRACE"))
    res = run_bass_kernel_spmd(nc, in_maps, list(range(NCORES)), trace=trace)
    LAST_RESULT = res

    acc = np.zeros((T, HIDDEN), dtype=np.float64)
    for c in range(NCORES):
        acc += res.results[c]["out"]
    out = (acc + b_o).astype(np.float32).reshape(B, S, HIDDEN)
    return out


# revision 14
# speedup vs baseline: 1.4138x; 1.1445x over previous
"""Multi-head attention (B=2, S=2048, H=1024, 16 heads x 64d) on 8 trn2 cores.

Sharding: tensor-parallel over heads (2 heads/core). Each core computes the
qkv projection for its 384 output features, attention for its 2 heads, and a
partial o_proj ([4096,1024] over its 128-feature slice). Host sums the 8
partials and adds b_o.

Device layout (per core, feature-major):
  QT/KT [128, 4096]  rows = head_local*64 + d, cols = b*2048 + s  (fp32r)
  S^T orientation for scores ([k, q]) so softmax-sum over k falls out of the
  PV matmul via a ones-column appended to V; exp runs on ScalarE from PSUM;
  normalization = reciprocal of the sums row + ones-broadcast matmul + mul.
All matmuls run as float32r (1 cyc/row vs 4 for fp32, ~1e-4 rel err).
"""
import sys

sys.path.insert(0, "/opt/trn_rl_repo")
import numpy as np

NHEADS = 16
HEAD_DIM = 64
HIDDEN = 1024
QKV = NHEADS * HEAD_DIM  # 1024
SCALING = HEAD_DIM ** -0.5
B = 2
S = 2048
T = B * S  # 4096
NCORES = 8
HPC = NHEADS // NCORES  # 2 heads per core
FEAT = HPC * HEAD_DIM  # 128
CHUNK = 512
NCHUNK = S // CHUNK  # 4 per batch
KSLABS = HIDDEN // 128  # 8
SSLABS = S // 128  # 16
D1 = HEAD_DIM + 1  # 65

_CACHE = {}
LAST_RESULT = None  # BassKernelResults of the most recent kernel() call


def _split_waits(nc, keep=1):
    """Hoist excess per-instruction sem waits into standalone EventSemaphore
    instructions (walrus codegen has small per-opcode wait budgets)."""
    import bass_rust
    import concourse.mybir as mybir

    n_new = 0
    for f in nc.m.functions:
        for blk in f.blocks:
            out = []
            changed = False
            for inst in blk.instructions:
                si = inst.sync_info
                waits = list(si.on_wait) if si is not None else []
                if len(waits) > keep:
                    excess = waits[: len(waits) - keep]
                    kept = waits[len(waits) - keep:]
                    for w in excess:
                        out.append(mybir.InstEventSemaphore(
                            name=f"{inst.name}-esw{n_new}",
                            engine=inst.engine,
                            sync_info=bass_rust.SyncInfo(on_wait=[w], on_update=[]),
                        ))
                        n_new += 1
                    inst.sync_info = bass_rust.SyncInfo(
                        on_wait=kept, on_update=list(si.on_update))
                    changed = True
                out.append(inst)
            if changed:
                blk.instructions = out
    return n_new


def _build(reps=1):
    import concourse.bass as bass
    import concourse.mybir as mybir
    import concourse.tile as tile
    from concourse.masks import make_identity

    f32 = mybir.dt.float32
    f32r = mybir.dt.float32r
    f16 = mybir.dt.float16
    Exp = mybir.ActivationFunctionType.Exp

    nc = bass.Bass()
    xT = nc.dram_tensor("xT", [HIDDEN, T], f16, kind="ExternalInput")
    wqkvT = nc.dram_tensor("wqkvT", [HIDDEN, 3 * FEAT], f16, kind="ExternalInput")
    bqkv = nc.dram_tensor("bqkv", [FEAT, 3], f32, kind="ExternalInput")
    woT = nc.dram_tensor("woT", [FEAT, HIDDEN], f16, kind="ExternalInput")
    out_d = nc.dram_tensor("out", [T, HIDDEN], f32, kind="ExternalOutput")

    with tile.TileContext(nc) as tc, nc.allow_low_precision(reason="fp32r matmuls"):
        with (
            tc.tile_pool(name="sing", bufs=1) as sing,
            tc.tile_pool(name="xp", bufs=2) as xp,
            tc.tile_pool(name="pp", bufs=2) as pp,
            tc.tile_pool(name="stg", bufs=4) as stg,
            tc.tile_pool(name="sm", bufs=2) as sm,
            tc.tile_pool(name="op", bufs=2) as op,
            tc.tile_pool(name="ps_mm", bufs=2, space="PSUM") as ps_mm,
            tc.tile_pool(name="ps_s", bufs=2, space="PSUM") as ps_s,
            tc.tile_pool(name="ps_o", bufs=2, space="PSUM") as ps_o,
        ):
            wq_sb = sing.tile([128, KSLABS, 3 * FEAT], f16, tag="wq")
            wo_sb = sing.tile([FEAT, HIDDEN], f16, tag="wo")
            bq_sb = sing.tile([FEAT, 3], f32, tag="bq")
            ident = sing.tile([128, 128], f32, tag="id")
            ones1 = sing.tile([1, HEAD_DIM], f32r, tag="on")
            QT = sing.tile([128, T], f16, tag="qt")
            KT = sing.tile([128, T], f16, tag="kt")
            VT = sing.tile([128, T], f32, tag="vt")
            OT = sing.tile([128, T], f16, tag="ot")
            Vaug = sing.tile([128, B, HPC, SSLABS, D1], f16, tag="va")

            nc.sync.dma_start(
                out=wq_sb, in_=wqkvT[:].rearrange("(s p) f -> p s f", p=128))
            nc.sync.dma_start(out=wo_sb, in_=woT[:])
            nc.sync.dma_start(out=bq_sb, in_=bqkv[:])
            make_identity(nc, ident)
            ones_f = stg.tile([1, HEAD_DIM], f32, tag="onf")
            nc.vector.memset(ones_f, 1.0)
            nc.vector.tensor_copy(ones1, ones_f)
            vst = stg.tile([128, B * HPC * SSLABS], f32, tag="vst")
            nc.vector.memset(vst, 1.0)
            nc.vector.tensor_copy(Vaug[:, :, :, :, HEAD_DIM:D1], vst)

            xT_r = xT[:].rearrange("(s p) t -> s p t", p=128)

            xT_c = xT[:].rearrange("(s p) t -> p s t", p=128)

            def qkv_chunk(b, n):
                g = b * NCHUNK + n
                lo, hi = g * CHUNK, (g + 1) * CHUNK
                xc = xp.tile([128, KSLABS, CHUNK], f16, tag="xc")
                nc.sync.dma_start(out=xc, in_=xT_c[:, :, lo:hi])
                for m, dest in enumerate((QT, KT, VT)):
                    acc = ps_mm.tile([128, CHUNK], f32, tag="mm")
                    for s in range(KSLABS):
                        nc.tensor.matmul(
                            acc, wq_sb[:, s, m * FEAT:(m + 1) * FEAT], xc[:, s, :],
                            start=(s == 0), stop=(s == KSLABS - 1))
                    nc.vector.tensor_scalar_add(
                        dest[:, lo:hi], acc, bq_sb[:, m:m + 1])

            def vtrans(b, n):
                # transpose V token-slabs 4n..4n+3 (one qkv chunk's worth)
                for h in range(HPC):
                    for k in range(4 * n, 4 * n + 4):
                        tp = ps_mm.tile([128, HEAD_DIM], f32, tag="mm")
                        nc.tensor.transpose(
                            tp,
                            VT[64 * h:64 * h + 64,
                               b * S + 128 * k: b * S + 128 * (k + 1)],
                            ident[64 * h:64 * h + 64, 64 * h:64 * h + 64])
                        nc.vector.tensor_copy(Vaug[:, b, h, k, 0:HEAD_DIM], tp)

            GRP = 2  # S-slabs per exp group (ps_s holds GRP banks x 2 bufs)

            def attn_unit(b, h, qc):
                qlo = b * S + qc * CHUNK
                qsl = slice(qlo, qlo + CHUNK)
                o_ps = ps_o.tile([D1, CHUNK], f32, tag="o")
                for grp in range(SSLABS // GRP):
                    s_ps = ps_s.tile([128, GRP, CHUNK], f32, tag="s")
                    for kk in range(GRP):
                        k = grp * GRP + kk
                        nc.tensor.matmul(
                            s_ps[:, kk, :],
                            KT[64 * h:64 * h + 64,
                               b * S + 128 * k: b * S + 128 * (k + 1)],
                            QT[64 * h:64 * h + 64, qsl],
                            start=True, stop=True)
                    pt = pp.tile([128, GRP, CHUNK], f16, tag="pt")
                    nc.scalar.activation(out=pt, in_=s_ps, func=Exp)
                    for kk in range(GRP):
                        k = grp * GRP + kk
                        nc.tensor.matmul(
                            o_ps, Vaug[:, b, h, k, :], pt[:, kk, :],
                            start=(k == 0), stop=(k == SSLABS - 1))
                rec = sm.tile([1, CHUNK], f32r, tag="rec")
                nc.vector.reciprocal(rec, o_ps[HEAD_DIM:D1, :])
                b_ps = ps_mm.tile([HEAD_DIM, CHUNK], f32, tag="mm")
                nc.tensor.matmul(b_ps, ones1, rec, start=True, stop=True)
                rb = sm.tile([HEAD_DIM, CHUNK], f32, tag="rb")
                nc.vector.tensor_copy(rb, b_ps)
                nc.vector.tensor_mul(
                    OT[64 * h:64 * h + 64, qsl], o_ps[0:HEAD_DIM, :], rb)

            def oproj_group(j):
                # token tiles 4j..4j+3 (tokens 512j..512j+512), one out-DMA
                ost = op.tile([128, 4, HIDDEN], f32, tag="ost")
                for jj in range(4):
                    t = 4 * j + jj
                    for nh in range(HIDDEN // CHUNK):
                        acc = ps_mm.tile([128, CHUNK], f32, tag="mm")
                        nc.tensor.matmul(
                            acc, OT[:, 128 * t:128 * (t + 1)],
                            wo_sb[:, nh * CHUNK:(nh + 1) * CHUNK],
                            start=True, stop=True)
                        nc.vector.tensor_copy(
                            ost[:, jj, nh * CHUNK:(nh + 1) * CHUNK], acc)
                nc.sync.dma_start(
                    out=out_d[512 * j:512 * (j + 1), :].rearrange(
                        "(jj p) h -> p jj h", p=128),
                    in_=ost)

            # ---- emission order: overlap qkv(b1) / oproj(b0) with attention ----
            for _rep in range(reps):
                for n in range(NCHUNK):
                    qkv_chunk(0, n)
                    vtrans(0, n)
                units_b0 = [(0, h, qc) for h in range(HPC) for qc in range(NCHUNK)]
                for i, (b, h, qc) in enumerate(units_b0):
                    attn_unit(b, h, qc)
                    if i < NCHUNK:
                        qkv_chunk(1, i)
                        vtrans(1, i)
                units_b1 = [(1, h, qc) for h in range(HPC) for qc in range(NCHUNK)]
                for i, (b, h, qc) in enumerate(units_b1):
                    attn_unit(b, h, qc)
                    if h == 0:             # batch-0 groups 0..3
                        oproj_group(qc)
                    else:                  # batch-1 group qc ready after both heads
                        oproj_group(4 + qc)

    _split_waits(nc)
    return nc


def kernel(hidden_states, w_qkv, b_qkv, w_o, b_o):
    global LAST_RESULT
    from concourse.bass_utils import run_bass_kernel_spmd
    import os

    if "nc" not in _CACHE:
        _CACHE["nc"] = _build()
    nc = _CACHE["nc"]

    x16 = np.ascontiguousarray(
        np.asarray(hidden_states, dtype=np.float32).reshape(T, HIDDEN).T
    ).astype(np.float16)
    w_qkv = np.asarray(w_qkv, dtype=np.float32)
    b_qkv = np.asarray(b_qkv, dtype=np.float32)
    w_o = np.asarray(w_o, dtype=np.float32)
    b_o = np.asarray(b_o, dtype=np.float32)

    in_maps = []
    for c in range(NCORES):
        rq = slice(c * FEAT, (c + 1) * FEAT)
        wq = w_qkv[0:QKV][rq] * SCALING
        wk = w_qkv[QKV:2 * QKV][rq]
        wv = w_qkv[2 * QKV:3 * QKV][rq]
        bq = b_qkv[0:QKV][rq] * SCALING
        bk = b_qkv[QKV:2 * QKV][rq]
        bv = b_qkv[2 * QKV:3 * QKV][rq]
        in_maps.append({
            "xT": x16,
            "wqkvT": np.ascontiguousarray(
                np.concatenate([wq, wk, wv], axis=0).T).astype(np.float16),
            "bqkv": np.ascontiguousarray(np.stack([bq, bk, bv], axis=1)),
            "woT": np.ascontiguousarray(w_o[:, rq].T).astype(np.float16),
        })

    trace = bool(os.environ.get("KERNEL_TRACE"))
    res = run_bass_kernel_spmd(nc, in_maps, list(range(NCORES)), trace=trace)
    LAST_RESULT = res

    acc = np.zeros((T, HIDDEN), dtype=np.float64)
    for c in range(NCORES):
        acc += res.results[c]["out"]
    out = (acc + b_o).astype(np.float32).reshape(B, S, HIDDEN)
    return out
